# revision 4
# baseline (speedup 1.0000x reference)
"""DTW (symmetric2, L1 cost) batch kernel for Trainium2, 8 NeuronCores.

Problem: 64 pairs of length-1024 fp32 sequences; per pair the full
1024x1024 DTW dynamic program; output = mean over pairs of
D[n-1, m-1] / (n + m).

Strategy per core (8 samples each):
  - Row-scan formulation: for each DP row,
        P[j]   = min(Dprev[j-1] + d[j], Dprev[j])
        D[j]   = min(P[j], D[j-1]) + d[j]
    The serial in-row recurrence maps onto the DVE tensor_tensor_scan
    instruction (op0=min, op1=add); the scan is widened to 65 columns
    with a BIG/0 leading element so out[0] passes the carry through,
    which doubles as next row's Dprev[-1] boundary (no copy needed).
  - Columns split into 16 chunks of 64; partition p = 8*chunk + sample.
    Chunks run in a software wavefront: chunk c processes row block
    i//R at macro-step tau = i//R + 2c (double skew gives the boundary
    transfer a full macro-step of slack).
  - The chunk->chunk boundary columns (a +8 partition shift, illegal
    for DVE access patterns) move through the TensorEngine once per
    macro-step: one [128x128]@[128x8] matmul with a shift matrix, plus
    a second accumulating matmul that writes BIG into chunk 0's rows.
  - Local cost rows d[j] = |x_i - y_j| are produced by the Scalar
    (activation) engine off the critical path.
  - Row state lives in a 16-slot rotating arena so the matmul reads
    all 8 right-boundary columns with one strided access pattern.
"""

import sys

sys.path.insert(0, "/opt/trn_rl_repo")

import numpy as np

import concourse.bass as bass
import concourse.bacc as bacc
import concourse.mybir as mybir
from concourse import tile
from concourse.bass_utils import run_bass_kernel_spmd

AF = mybir.ActivationFunctionType
ALU = mybir.AluOpType
FP32 = mybir.dt.float32

NCORES = 8
B = 8             # samples per core
N = 1024          # sequence length (rows == cols)
C = 16            # column chunks
W = N // C        # 64 columns per chunk
R = 8             # rows per macro-step
SKEW = 2          # macro-steps of lag between adjacent chunks
T = N // R + SKEW * (C - 1)   # 158 macro-steps
S_TOTAL = T * R               # 1264 row-steps
NSLOT = 16                    # row-state arena slots
BIG = 1.0e30

_CACHE = {}


def _build():
    nc = bacc.Bacc("TRN2", target_bir_lowering=False, debug=False)
    x8 = nc.declare_dram_parameter("x8", [B, N], FP32, isOutput=False)
    y8 = nc.declare_dram_parameter("y8", [B, N], FP32, isOutput=False)
    s8in = nc.declare_dram_parameter("s8", [128, 128], FP32, isOutput=False)
    out = nc.declare_dram_parameter("dists", [B, 1], FP32, isOutput=True)

    with tile.TileContext(nc) as tc:
        with (
            tc.tile_pool(name="persist", bufs=1) as pp,
            tc.tile_pool(name="qpool", bufs=2) as qpool,
            tc.tile_pool(name="psum", bufs=2, space=bass.MemorySpace.PSUM) as psp,
        ):
            S8 = pp.tile([128, 128], FP32, tag="s8t")
            W2 = pp.tile([128, 128], FP32, tag="w2t")
            ONES = pp.tile([128, R], FP32, tag="ones")
            Y = pp.tile([128, W], FP32, tag="y")
            XS = pp.tile([128, S_TOTAL], FP32, tag="xs")
            BB = pp.tile([128, NSLOT, W + 1], FP32, tag="bb")
            LF = [
                pp.tile([128, R + 1], FP32, name=f"lft{i}", tag=f"lf{i}")
                for i in range(3)
            ]
            ZC = pp.tile([128, 1], FP32, tag="zc")

            nc.sync.dma_start(S8[:], s8in[:])
            # X skew: XS[8c+b, s] = x[b, s - SKEW*R*c]; pad BIG so
            # out-of-range rows produce huge local costs (+inf rows).
            nc.vector.memset(XS[:], BIG)
            for c in range(C):
                o = SKEW * R * c
                nc.sync.dma_start(XS[8 * c : 8 * c + 8, o : o + N], x8[:])
                nc.sync.dma_start(Y[8 * c : 8 * c + 8, :], y8[:, W * c : W * c + W])
            # negate in place: bias for |y - x| activation
            nc.vector.tensor_scalar_mul(XS[:], XS[:], -1.0)

            nc.vector.memset(BB[:], BIG)
            for i in range(3):
                nc.vector.memset(LF[i][:], BIG)
            nc.vector.memset(ZC[:], BIG)
            nc.vector.memset(ZC[0:8, :], 0.0)
            # patch matmul: W2.T @ ONES adds BIG into partitions 0:8
            nc.vector.memset(W2[:], 0.0)
            nc.vector.memset(W2[0:1, 0:8], BIG)
            nc.vector.memset(ONES[:], 1.0)

            # d tiles [128, W+1]: col 0 stays 0 forever (scan leading
            # element); ACT writes cols 1..W each row. P tiles keep
            # col 0 = BIG forever.
            dts = [
                pp.tile([128, W + 1], FP32, name=f"dt{i}", tag=f"dt{i}")
                for i in range(4)
            ]
            pts = [
                pp.tile([128, W + 1], FP32, name=f"pt{i}", tag=f"pt{i}")
                for i in range(3)
            ]
            for t_ in dts:
                nc.vector.memset(t_[:, 0:1], 0.0)
            for t_ in pts:
                nc.vector.memset(t_[:, 0:1], BIG)

            for tau in range(T):
                lf_cur = LF[tau % 3]
                lf_nxt = LF[(tau + 2) % 3]
                lf_mid = LF[(tau + 1) % 3]
                do_mm = tau < T - SKEW
                if do_mm:
                    acc = psp.tile([128, R], FP32, tag="acc", name="acc")
                for r in range(R):
                    s = R * tau + r
                    b_prev = BB[:, (s - 1) % NSLOT, :]
                    b_cur = BB[:, s % NSLOT, :]
                    d = dts[s % 4]
                    nc.scalar.activation(
                        d[:, 1 : W + 1],
                        Y[:],
                        AF.Abs,
                        bias=XS[:, s : s + 1],
                        scale=1.0,
                    )
                    q = qpool.tile([128, W], FP32, tag="q", name="q")
                    nc.vector.tensor_tensor(
                        q[:], b_prev[:, 0:W], d[:, 1 : W + 1], op=ALU.add
                    )
                    p = pts[s % 3]
                    nc.vector.tensor_tensor(
                        p[:, 1 : W + 1], q[:], b_prev[:, 1 : W + 1], op=ALU.min
                    )
                    if s == 0:
                        # special: scan cols 1..W with zero-carry for chunk 0;
                        # boundary col written separately
                        nc.vector.tensor_tensor_scan(
                            b_cur[:, 1 : W + 1],
                            p[:, 1 : W + 1],
                            d[:, 1 : W + 1],
                            ZC[:, 0:1],
                            op0=ALU.min,
                            op1=ALU.add,
                        )
                        nc.vector.memset(b_cur[:, 0:1], BIG)
                    else:
                        # 65-wide scan: out[0] = carry (data0 BIG, data1 0),
                        # doubling as next row's Dprev[-1]
                        nc.vector.tensor_tensor_scan(
                            b_cur[:, 0 : W + 1],
                            p[:, 0 : W + 1],
                            d[:, 0 : W + 1],
                            lf_cur[:, r + 1 : r + 2],
                            op0=ALU.min,
                            op1=ALU.add,
                        )
                if do_mm:
                    k0 = (R * tau) % NSLOT
                    nc.tensor.matmul(
                        acc[:, 0:R],
                        S8[:],
                        BB[:, k0 : k0 + R, W],
                        start=True,
                        stop=False,
                    )
                    nc.tensor.matmul(
                        acc[:, 0:R],
                        W2[:],
                        ONES[:],
                        start=False,
                        stop=True,
                        skip_group_check=True,
                    )
                    nc.vector.tensor_copy(lf_nxt[:, 1 : R + 1], acc[:, 0:R])
                    nc.vector.tensor_copy(lf_nxt[:, 0:1], lf_mid[:, R : R + 1])

            last_cur = BB[:, (S_TOTAL - 1) % NSLOT, :]
            nc.sync.dma_start(out[:], last_cur[120:128, W : W + 1])

    nc.compile()
    return nc


def _shift_matrix():
    s8 = np.zeros((128, 128), np.float32)
    for r in range(120):
        s8[r, r + 8] = 1.0  # out[p] = in[p - 8]
    return s8


def kernel(x: np.ndarray, x_target: np.ndarray) -> np.ndarray:
    x = np.ascontiguousarray(np.asarray(x, np.float32))
    y = np.ascontiguousarray(np.asarray(x_target, np.float32))
    if "nc" not in _CACHE:
        _CACHE["nc"] = _build()
    nc = _CACHE["nc"]
    s8 = _shift_matrix()
    in_maps = [
        {"x8": x[8 * k : 8 * k + 8], "y8": y[8 * k : 8 * k + 8], "s8": s8}
        for k in range(NCORES)
    ]
    res = run_bass_kernel_spmd(nc, in_maps, list(range(NCORES))).results
    dists = np.concatenate([r["dists"][:, 0] for r in res]).astype(np.float32)
    dists = dists / np.float32(2.0 * N)
    return np.float32(np.mean(dists))
